# revision 12
# baseline (speedup 1.0000x reference)
"""Otsu-threshold binary region proposal kernel for Trainium2 (8 NeuronCores).

Algorithm (per image of 224*224 pixels, 512 images total, data-parallel over
8 cores / 64 images per core):

  reference:  cam = floor(x*255); per-image 256-bin histogram; Otsu threshold
              via argmax of inter-class variance restricted to [vmin, vmax);
              roi = (cam > th), 0 for degenerate images.

Device pass A (histogram):
  A 256-bin histogram is too expensive elementwise, so we use a thermometer
  decomposition: with hi = cam >> 4, lo = cam & 15,
      R[tau, sigma] = sum_p colA_tau(p) * colB_sigma(p)
  where colA_tau ~ [hi >= tau] and colB_sigma ~ [lo >= sigma] are built on
  DVE / ACT / GPSIMD (16+16 cut columns, bf16), and the 16x16 pair-count
  matrix is accumulated on the TensorEngine (one [128,16]x[128,16] matmul per
  128-pixel chunk into PSUM).  ACT's columns are +-1 coded (Sign), DVE /
  GPSIMD's are 0/1 coded (is_le); the host decodes mixed codings exactly via
  the marginal row/col (tau=0 / sigma=0 are always-true cuts).
  All counts are exact small integers in fp32/bf16.

Host (exact float32, mirrors jax reference op-for-op):
  W -> 2D difference -> 256-bin histogram -> cumsums -> inter-class variance
  -> argmax -> threshold; then fold "cam > th" into a single fp32 cut on raw
  x (monotonicity of x -> floor(fl(255x)) makes this exact).

Device pass B (mask): mask = (x >= dth_image) as uint8, streamed at memory
speed.  Host casts to int32.

floor() trick (no floor ALU op): negcam = fmod(255x, 1) - 255x = -floor(255x)
exactly in fp32; neglo = fmod(negcam, 16) = -(cam mod 16).  Comparisons then
use is_le against negated cuts; integers up to 255 are exact in bf16.
"""

import math
import os
import sys

import numpy as np

sys.path.insert(0, "/opt/trn_rl_repo")

import concourse.bacc as bacc
import concourse.bass as bass
import concourse.mybir as mybir
from concourse.bass_utils import run_bass_kernel_spmd
from concourse.tile import TileContext

# ---------------------------------------------------------------------------
# Problem geometry (hardcoded per spec)
B, N, H, W_IMG = 64, 8, 224, 224
PIX = H * W_IMG              # 50176
PARTS = 128
CPI = PIX // PARTS           # 392 chunks (columns) per image
N_CORES = 8
IMGS_PER_CORE = (B // N_CORES) * N      # 64
NBINS = 256

# Tunables
GROUP = 3          # images per thermo group (ACT instruction batching)
PSUM_G = 8         # images per PSUM tile ([16, 16*PSUM_G])
# Cut assignment: plane A rows tau=0..15 (cut on cam at 16*tau, DVE is_ge,
# 0/1 coded), plane B rows sigma=0..15 (cut on lo at sigma).  B rows below
# ACT_B_START are DVE is_ge (0/1); the rest are ACT Sign (+-1 coded).
ACT_B_START = 6

FP32 = mybir.dt.float32
BF16 = mybir.dt.bfloat16
I16 = mybir.dt.int16
I8 = mybir.dt.int8
U8 = mybir.dt.uint8
ALU = mybir.AluOpType
ACTF = mybir.ActivationFunctionType
MAGIC = 8388608.0  # 2^23: fp32 ulp 1.0 => fl(s + MAGIC) = MAGIC + round(s)


def _enc_pm():
    """Which rows are +-1 coded (ACT Sign)."""
    encA = np.zeros(16, dtype=bool)
    encB = np.zeros(16, dtype=bool)
    encB[ACT_B_START:] = True
    return encA, encB


# ---------------------------------------------------------------------------
# Pass A: histogram kernel
def build_hist_nc(nimg=IMGS_PER_CORE, cpi=CPI, group=GROUP, psum_g=PSUM_G):
    nc = bacc.Bacc("TRN2", target_bir_lowering=False, debug=False)
    x_d = nc.dram_tensor("x", [nimg, PARTS, cpi], FP32, kind="ExternalInput")
    w_d = nc.dram_tensor("w_raw", [16, 16 * nimg], FP32, kind="ExternalOutput")

    with TileContext(nc) as tc:
        with (
            tc.tile_pool(name="const", bufs=1) as cpool,
            tc.tile_pool(name="xin", bufs=2) as xpool,
            tc.tile_pool(name="prep", bufs=2) as ppool,
            tc.tile_pool(name="thermo", bufs=2) as tpool,
            tc.tile_pool(name="psum", bufs=2, space="PSUM") as qpool,
        ):
            stage = cpool.tile([16, 16 * nimg], FP32, tag="stage")
            nACT = 16 - ACT_B_START
            act_bias = cpool.tile([PARTS, nACT], FP32, tag="abias")
            for j, sg in enumerate(range(ACT_B_START, 16)):
                nc.vector.memset(act_bias[:, j:j + 1], 0.5 - sg)

            n_groups = math.ceil(nimg / group)
            psum_t = None
            for g in range(n_groups):
                g0 = g * group
                g1 = min(g0 + group, nimg)
                gw = (g1 - g0) * cpi

                x_t = xpool.tile([PARTS, group * cpi], FP32, tag="x")
                rM = ppool.tile([PARTS, group * cpi], FP32, tag="rM")
                c1 = ppool.tile([PARTS, group * cpi], I8, tag="c1")
                ci = ppool.tile([PARTS, group * cpi], I16, tag="ci")
                lo = ppool.tile([PARTS, group * cpi], I16, tag="lo")
                A_t = tpool.tile([PARTS, 16, group * cpi], BF16, tag="A")
                B_t = tpool.tile([PARTS, 16, group * cpi], BF16, tag="B")

                for i in range(g0, g1):
                    nc.sync.dma_start(
                        out=x_t[:, (i - g0) * cpi:(i - g0 + 1) * cpi],
                        in_=x_d.ap()[i],
                    )

                # ACT: s = fl(255x) in-place on x; rM = fl(s + 2^23) = 2^23 + round(s)
                nc.scalar.activation(
                    out=x_t[:, :gw], in_=x_t[:, :gw],
                    func=ACTF.Copy, bias=0.0, scale=255.0,
                )
                nc.scalar.activation(
                    out=rM[:, :gw], in_=x_t[:, :gw],
                    func=ACTF.Copy, bias=MAGIC, scale=1.0,
                )
                # c1 = [round(s) > s];  cam = round(s) - c1 = floor(s) (int16)
                nc.vector.scalar_tensor_tensor(
                    out=c1[:, :gw], in0=rM[:, :gw], scalar=-MAGIC,
                    in1=x_t[:, :gw], op0=ALU.add, op1=ALU.is_gt,
                )
                nc.vector.scalar_tensor_tensor(
                    out=ci[:, :gw], in0=rM[:, :gw], scalar=-MAGIC,
                    in1=c1[:, :gw], op0=ALU.add, op1=ALU.subtract,
                )
                nc.vector.tensor_scalar(
                    out=lo[:, :gw], in0=ci[:, :gw],
                    scalar1=15, scalar2=None, op0=ALU.bitwise_and,
                )

                # thermo columns (immediate-scalar compares run at DVE 4x)
                for tau in range(16):
                    nc.vector.tensor_scalar(
                        out=A_t[:, tau, :gw], in0=ci[:, :gw],
                        scalar1=16 * tau, scalar2=None, op0=ALU.is_ge,
                    )
                for sg in range(ACT_B_START):
                    nc.vector.tensor_scalar(
                        out=B_t[:, sg, :gw], in0=lo[:, :gw],
                        scalar1=sg, scalar2=None, op0=ALU.is_ge,
                    )
                # ACT: B rows, +-1 coded: sign(lo - sigma + 0.5)
                for j, sg in enumerate(range(ACT_B_START, 16)):
                    nc.scalar.activation(
                        out=B_t[:, sg, :gw], in_=lo[:, :gw],
                        func=ACTF.Sign, bias=act_bias[:, j:j + 1], scale=1.0,
                    )

                # PE: accumulate R = A^T B per image
                for i in range(g0, g1):
                    il = i - g0
                    if i % psum_g == 0:
                        psum_t = qpool.tile([16, 16 * psum_g], FP32, tag="ps")
                    pwin = psum_t[:, 16 * (i % psum_g):16 * (i % psum_g) + 16]
                    for c in range(cpi):
                        nc.tensor.matmul(
                            pwin,
                            A_t[:, :, il * cpi + c],
                            B_t[:, :, il * cpi + c],
                            start=(c == 0),
                            stop=(c == cpi - 1),
                        )
                    if i % psum_g == psum_g - 1 or i == nimg - 1:
                        lo_i = (i // psum_g) * psum_g
                        nc.vector.tensor_copy(
                            out=stage[:, 16 * lo_i:16 * (i + 1)],
                            in_=psum_t[:, : 16 * (i - lo_i + 1)],
                        )
            nc.sync.dma_start(out=w_d.ap(), in_=stage[:])
    nc.finalize()
    return nc


# ---------------------------------------------------------------------------
# Pass B: mask kernel
def build_mask_nc(nimg=IMGS_PER_CORE, cpi=CPI):
    nc = bacc.Bacc("TRN2", target_bir_lowering=False, debug=False)
    x_d = nc.dram_tensor("x", [nimg, PARTS, cpi], FP32, kind="ExternalInput")
    t_d = nc.dram_tensor("dth", [nimg, PARTS], FP32, kind="ExternalInput")
    m_d = nc.dram_tensor("mask", [nimg, PARTS, cpi], U8, kind="ExternalOutput")

    with TileContext(nc) as tc:
        with (
            tc.tile_pool(name="xin", bufs=4) as xpool,
            tc.tile_pool(name="th", bufs=4) as tpool,
            tc.tile_pool(name="mo", bufs=4) as mpool,
        ):
            for i in range(nimg):
                x_t = xpool.tile([PARTS, cpi], FP32, tag="x")
                th_t = tpool.tile([PARTS, 1], FP32, tag="t")
                m_t = mpool.tile([PARTS, cpi], U8, tag="m")
                nc.sync.dma_start(out=x_t[:], in_=x_d.ap()[i])
                nc.sync.dma_start(out=th_t[:], in_=t_d.ap()[i].unsqueeze(1))
                nc.vector.tensor_scalar(
                    out=m_t[:], in0=x_t[:], scalar1=th_t[:],
                    scalar2=None, op0=ALU.is_ge,
                )
                nc.sync.dma_start(out=m_d.ap()[i], in_=m_t[:])
    nc.finalize()
    return nc


# ---------------------------------------------------------------------------
# Host: decode W, exact-float32 Otsu, threshold folding
def decode_hist(w_raw, nimg=IMGS_PER_CORE, npix=PIX):
    """w_raw [16, 16*nimg] fp32 -> hist [nimg, 256] int64 (exact)."""
    encA, encB = _enc_pm()
    R = np.round(np.asarray(w_raw, np.float64)).astype(np.int64)
    R = R.reshape(16, nimg, 16).transpose(1, 0, 2)  # [img, tau, sigma]
    P = npix
    # marginals from always-true rows (tau=0 / sigma=0 columns are exact ones)
    sumB = np.where(encB[None, :], (R[:, 0, :] + P) // 2, R[:, 0, :])  # [img,16]
    sumA = np.where(encA[None, :], (R[:, :, 0] + P) // 2, R[:, :, 0])  # [img,16]
    eA = encA[None, :, None]
    eB = encB[None, None, :]
    sA = sumA[:, :, None]
    sB = sumB[:, None, :]
    W = np.where(
        ~eA & ~eB, R,
        np.where(
            eA & ~eB, (R + sB) // 2,
            np.where(~eA & eB, (R + sA) // 2, (R + 2 * sA + 2 * sB - P) // 4),
        ),
    )
    # sanity: the integer divisions above must be exact
    chk = np.where(
        ~eA & ~eB, 0,
        np.where(eA & ~eB, (R + sB) % 2,
                 np.where(~eA & eB, (R + sA) % 2, (R + 2 * sA + 2 * sB - P) % 4)),
    )
    assert not chk.any(), "non-integer decode: device histogram corrupted"
    Wp = np.zeros((nimg, 17, 17), np.int64)
    Wp[:, :16, :16] = W
    hist = (Wp[:, :16, :16] - Wp[:, 1:, :16] - Wp[:, :16, 1:] + Wp[:, 1:, 1:])
    hist = hist.reshape(nimg, 256)
    assert (hist >= 0).all() and (hist.sum(1) == P).all(), "bad histogram"
    return hist


def otsu_f32(hist):
    """Mirror the jax float32 reference exactly. hist [n, 256] int64 -> th int, bad mask."""
    f = hist.astype(np.float32)
    centers = np.arange(NBINS, dtype=np.float32)
    w1 = np.cumsum(f, axis=1, dtype=np.float32)
    total = w1[:, -1:]
    s1 = np.cumsum(f * centers, axis=1, dtype=np.float32)
    stot = s1[:, -1:]
    w2 = total - w1
    with np.errstate(divide="ignore", invalid="ignore"):
        m1 = s1 / w1
        m2 = (stot - s1) / w2
        d = m1 - m2
        var12 = (w1 * w2) * (d * d)
    nz = hist > 0
    t = np.arange(NBINS)
    vmin = np.argmax(nz, axis=1)
    vmax = NBINS - 1 - np.argmax(nz[:, ::-1], axis=1)
    valid = (t[None, :] >= vmin[:, None]) & (t[None, :] < vmax[:, None])
    var12 = np.where(valid, var12, np.float32(-1.0))
    th = np.argmax(var12, axis=1)
    th = np.where(th == 0, 1, th)
    th = np.where(th == 255, 254, th)
    bad = vmin == vmax
    return th, bad


def _min_x_for_cut(c):
    """Smallest fp32 x with fl(255*x) >= c (c integer 1..255)."""
    f255 = np.float32(255.0)
    d = np.float32(np.float64(c) / 255.0)
    # walk down while still satisfying, then ensure satisfied
    for _ in range(8):
        dn = np.nextafter(d, np.float32(-1.0), dtype=np.float32)
        if np.float32(f255 * dn) >= c:
            d = dn
        else:
            break
    while np.float32(f255 * d) < c:
        d = np.nextafter(d, np.float32(2.0), dtype=np.float32)
    return d


_CUT_TABLE = None


def cut_table():
    global _CUT_TABLE
    if _CUT_TABLE is None:
        _CUT_TABLE = np.array(
            [np.float32(0.0)] + [_min_x_for_cut(c) for c in range(1, 256)],
            dtype=np.float32,
        )
    return _CUT_TABLE


def thresholds_to_cuts(th, bad):
    """mask = (cam > th) == (x >= dth); degenerate images -> never."""
    tab = cut_table()
    dth = tab[np.asarray(th) + 1]
    return np.where(bad, np.float32(2.0), dth).astype(np.float32)


# ---------------------------------------------------------------------------
_NC_CACHE = {}


def _get_ncs():
    if "hist" not in _NC_CACHE:
        _NC_CACHE["hist"] = build_hist_nc()
        _NC_CACHE["mask"] = build_mask_nc()
    return _NC_CACHE["hist"], _NC_CACHE["mask"]


def kernel(x: np.ndarray, _profile: dict | None = None) -> np.ndarray:
    x = np.ascontiguousarray(np.asarray(x, dtype=np.float32))
    assert x.shape == (B, N, H, W_IMG)
    nc_hist, nc_mask = _get_ncs()

    bpc = B // N_CORES
    shards = [
        np.ascontiguousarray(
            x[k * bpc:(k + 1) * bpc].reshape(IMGS_PER_CORE, PARTS, CPI)
        )
        for k in range(N_CORES)
    ]
    core_ids = list(range(N_CORES))

    kwargs_a = dict(_profile.get("a", {})) if _profile else {}
    res_a = run_bass_kernel_spmd(
        nc_hist, [{"x": s} for s in shards], core_ids=core_ids, **kwargs_a
    )
    if _profile is not None:
        _profile["res_a"] = res_a

    dths = []
    for k in range(N_CORES):
        hist = decode_hist(res_a.results[k]["w_raw"])
        th, bad = otsu_f32(hist)
        dth = thresholds_to_cuts(th, bad)
        dths.append(np.repeat(dth[:, None], PARTS, axis=1).astype(np.float32))

    kwargs_b = dict(_profile.get("b", {})) if _profile else {}
    res_b = run_bass_kernel_spmd(
        nc_mask,
        [{"x": s, "dth": d} for s, d in zip(shards, dths)],
        core_ids=core_ids,
        **kwargs_b,
    )
    if _profile is not None:
        _profile["res_b"] = res_b

    out = np.empty((B, N, H, W_IMG), np.int32)
    for k in range(N_CORES):
        m = res_b.results[k]["mask"]  # [64, 128, 392] u8
        out[k * bpc:(k + 1) * bpc] = (
            m.reshape(bpc, N, H, W_IMG).astype(np.int32)
        )
    return out


# revision 17
# speedup vs baseline: 2.1360x; 2.1360x over previous
"""Otsu-threshold binary region proposal kernel for Trainium2 (8 NeuronCores).

Algorithm (per image of 224*224 pixels, 512 images total, data-parallel over
8 cores / 64 images per core):

  reference:  cam = floor(x*255); per-image 256-bin histogram; Otsu threshold
              via argmax of inter-class variance restricted to [vmin, vmax);
              roi = (cam > th), 0 for degenerate images.

Device pass A (histogram):
  A 256-bin histogram is too expensive elementwise, so we use a thermometer
  decomposition: with hi = cam >> 4, lo = cam & 15,
      R[tau, sigma] = sum_p colA_tau(p) * colB_sigma(p)
  where colA_tau ~ [hi >= tau] and colB_sigma ~ [lo >= sigma] are built on
  DVE / ACT / GPSIMD (16+16 cut columns, bf16), and the 16x16 pair-count
  matrix is accumulated on the TensorEngine (one [128,16]x[128,16] matmul per
  128-pixel chunk into PSUM).  ACT's columns are +-1 coded (Sign), DVE /
  GPSIMD's are 0/1 coded (is_le); the host decodes mixed codings exactly via
  the marginal row/col (tau=0 / sigma=0 are always-true cuts).
  All counts are exact small integers in fp32/bf16.

Host (exact float32, mirrors jax reference op-for-op):
  W -> 2D difference -> 256-bin histogram -> cumsums -> inter-class variance
  -> argmax -> threshold; then fold "cam > th" into a single fp32 cut on raw
  x (monotonicity of x -> floor(fl(255x)) makes this exact).

Device pass B (mask): mask = (x >= dth_image) as uint8, streamed at memory
speed.  Host casts to int32.

floor() trick (no floor ALU op): negcam = fmod(255x, 1) - 255x = -floor(255x)
exactly in fp32; neglo = fmod(negcam, 16) = -(cam mod 16).  Comparisons then
use is_le against negated cuts; integers up to 255 are exact in bf16.
"""

import math
import os
import sys

import numpy as np

sys.path.insert(0, "/opt/trn_rl_repo")

import concourse.bacc as bacc
import concourse.bass as bass  # noqa: F401
import concourse.mybir as mybir
from concourse.bass_utils import run_bass_kernel_spmd
from concourse.tile import TileContext

# ---------------------------------------------------------------------------
# Problem geometry (hardcoded per spec)
B, N, H, W_IMG = 64, 8, 224, 224
PIX = H * W_IMG              # 50176
PARTS = 128
CPI = PIX // PARTS           # 392 chunks (columns) per image
N_CORES = 8
IMGS_PER_CORE = (B // N_CORES) * N      # 64
NBINS = 256

# Tunables
GROUP = 3          # images per thermo group (ACT instruction batching)
PSUM_G = 8         # images per PSUM tile ([16, 16*PSUM_G])
# Cut assignment: plane A rows tau=0..15 (cut on cam at 16*tau, DVE is_ge,
# 0/1 coded), plane B rows sigma=0..15 (cut on lo at sigma).  B rows below
# ACT_B_START are DVE is_ge (0/1); the rest are ACT Sign (+-1 coded).
ACT_B_START = 6

FP32 = mybir.dt.float32
BF16 = mybir.dt.bfloat16
I16 = mybir.dt.int16
I8 = mybir.dt.int8
U8 = mybir.dt.uint8
ALU = mybir.AluOpType
ACTF = mybir.ActivationFunctionType
MAGIC = 8388608.0  # 2^23: fp32 ulp 1.0 => fl(s + MAGIC) = MAGIC + round(s)


def _enc_pm():
    """Which rows are +-1 coded (ACT Sign)."""
    encA = np.zeros(16, dtype=bool)
    encB = np.zeros(16, dtype=bool)
    encB[ACT_B_START:] = True
    return encA, encB


# ---------------------------------------------------------------------------
# Pass A: histogram kernel
def build_hist_nc(nimg=IMGS_PER_CORE, cpi=CPI, group=GROUP, psum_g=PSUM_G):
    """Pass A.  Thermo tiles are laid out [128, W/8, 16, 8] so that each
    8-chunk pack is one contiguous [128, 128] block: the PE then runs one
    [128,128]x[128,128] matmul per 8 chunks (block-diagonal trick — psum row
    8*tau+c', col 8*sigma+c''; only c'==c'' blocks are meaningful and the
    host sums them).  N=16 matmuls were drain-bound at ~47ns; packed N=128
    matmuls measure ~69ns for 8x the work."""
    assert cpi % 8 == 0
    nc = bacc.Bacc("TRN2", target_bir_lowering=False, debug=False)
    x_d = nc.dram_tensor("x", [nimg, PARTS, cpi], FP32, kind="ExternalInput")
    w_d = nc.dram_tensor("w_raw", [nimg, PARTS, PARTS], FP32, kind="ExternalOutput")

    with TileContext(nc) as tc:
        with (
            tc.tile_pool(name="const", bufs=1) as cpool,
            tc.tile_pool(name="xin", bufs=2) as xpool,
            tc.tile_pool(name="prep", bufs=2) as ppool,
            tc.tile_pool(name="thermo", bufs=2) as tpool,
            tc.tile_pool(name="psum", bufs=4, space="PSUM") as qpool,
        ):
            nACT = 16 - ACT_B_START
            act_bias = cpool.tile([PARTS, nACT], FP32, tag="abias")
            for j, sg in enumerate(range(ACT_B_START, 16)):
                nc.vector.memset(act_bias[:, j:j + 1], 0.5 - sg)

            n_groups = math.ceil(nimg / group)
            for g in range(n_groups):
                g0 = g * group
                g1 = min(g0 + group, nimg)
                gw = (g1 - g0) * cpi
                gw8 = gw // 8

                x_t = xpool.tile([PARTS, group * cpi], FP32, tag="x")
                rM = ppool.tile([PARTS, group * cpi], FP32, tag="rM")
                c1 = ppool.tile([PARTS, group * cpi], I8, tag="c1")
                ci = ppool.tile([PARTS, group * cpi], I16, tag="ci")
                lo = ppool.tile([PARTS, group * cpi], I16, tag="lo")
                A_t = tpool.tile([PARTS, group * cpi // 8, 16, 8], BF16, tag="A")
                B_t = tpool.tile([PARTS, group * cpi // 8, 16, 8], BF16, tag="B")

                # one batched load for the whole group
                nc.sync.dma_start(
                    out=x_t[:, :gw].rearrange("p (i c) -> p i c", c=cpi),
                    in_=bass.AP(
                        x_d, g0 * PARTS * cpi,
                        [[cpi, PARTS], [PARTS * cpi, g1 - g0], [1, cpi]],
                    ),
                )

                # ACT: s = fl(255x) in-place on x; rM = fl(s + 2^23) = 2^23 + round(s)
                nc.scalar.activation(
                    out=x_t[:, :gw], in_=x_t[:, :gw],
                    func=ACTF.Copy, bias=0.0, scale=255.0,
                )
                nc.scalar.activation(
                    out=rM[:, :gw], in_=x_t[:, :gw],
                    func=ACTF.Copy, bias=MAGIC, scale=1.0,
                )
                # c1 = [round(s) > s];  cam = round(s) - c1 = floor(s) (int16)
                nc.vector.scalar_tensor_tensor(
                    out=c1[:, :gw], in0=rM[:, :gw], scalar=-MAGIC,
                    in1=x_t[:, :gw], op0=ALU.add, op1=ALU.is_gt,
                )
                nc.vector.scalar_tensor_tensor(
                    out=ci[:, :gw], in0=rM[:, :gw], scalar=-MAGIC,
                    in1=c1[:, :gw], op0=ALU.add, op1=ALU.subtract,
                )
                nc.vector.tensor_scalar(
                    out=lo[:, :gw], in0=ci[:, :gw],
                    scalar1=15, scalar2=None, op0=ALU.bitwise_and,
                )

                ci_v = ci[:, :gw].rearrange("p (a b) -> p a b", b=8)
                lo_v = lo[:, :gw].rearrange("p (a b) -> p a b", b=8)
                # thermo columns (immediate-scalar compares run at DVE 4x)
                for tau in range(16):
                    nc.vector.tensor_scalar(
                        out=A_t[:, :gw8, tau, :], in0=ci_v,
                        scalar1=16 * tau, scalar2=None, op0=ALU.is_ge,
                    )
                for sg in range(ACT_B_START):
                    nc.vector.tensor_scalar(
                        out=B_t[:, :gw8, sg, :], in0=lo_v,
                        scalar1=sg, scalar2=None, op0=ALU.is_ge,
                    )
                # ACT: B rows, +-1 coded: sign(lo - sigma + 0.5)
                for j, sg in enumerate(range(ACT_B_START, 16)):
                    nc.scalar.activation(
                        out=B_t[:, :gw8, sg, :], in_=lo_v,
                        func=ACTF.Sign, bias=act_bias[:, j:j + 1], scale=1.0,
                    )

                # PE: per image, 49 packed [128,128] matmuls accumulate in PSUM
                packs_per_img = cpi // 8
                for i in range(g0, g1):
                    il = i - g0
                    psum_t = qpool.tile([PARTS, PARTS], FP32, tag="ps")
                    for k in range(packs_per_img):
                        p = il * packs_per_img + k
                        nc.tensor.matmul(
                            psum_t[:],
                            A_t[:, p, :, :].rearrange("p a b -> p (a b)"),
                            B_t[:, p, :, :].rearrange("p a b -> p (a b)"),
                            start=(k == 0),
                            stop=(k == packs_per_img - 1),
                        )
                    w_sb = ppool.tile([PARTS, PARTS], FP32, tag="wsb")
                    nc.scalar.copy(w_sb[:], psum_t[:])
                    nc.sync.dma_start(out=w_d.ap()[i], in_=w_sb[:])
    nc.finalize()
    return nc


# ---------------------------------------------------------------------------
# Pass B: mask kernel
def build_mask_nc(nimg=IMGS_PER_CORE, cpi=CPI, mgroup=8):
    nc = bacc.Bacc("TRN2", target_bir_lowering=False, debug=False)
    x_d = nc.dram_tensor("x", [nimg, PARTS, cpi], FP32, kind="ExternalInput")
    t_d = nc.dram_tensor("dth", [nimg, PARTS], FP32, kind="ExternalInput")
    m_d = nc.dram_tensor("mask", [nimg, PARTS, cpi], U8, kind="ExternalOutput")

    with TileContext(nc) as tc:
        with (
            tc.tile_pool(name="cst", bufs=1) as cpool,
            tc.tile_pool(name="xin", bufs=3) as xpool,
            tc.tile_pool(name="mo", bufs=3) as mpool,
        ):
            # all thresholds in one transfer: sbuf[p, i] = dth[i, p]
            th_all = cpool.tile([PARTS, nimg], FP32, tag="t")
            nc.sync.dma_start(
                out=th_all[:],
                in_=bass.AP(t_d, 0, [[1, PARTS], [PARTS, nimg]]),
            )
            for g0 in range(0, nimg, mgroup):
                g1 = min(g0 + mgroup, nimg)
                x_t = xpool.tile([PARTS, mgroup, cpi], FP32, tag="x")
                m_t = mpool.tile([PARTS, mgroup, cpi], U8, tag="m")
                nc.sync.dma_start(
                    out=x_t[:, :g1 - g0, :],
                    in_=bass.AP(
                        x_d, g0 * PARTS * cpi,
                        [[cpi, PARTS], [PARTS * cpi, g1 - g0], [1, cpi]],
                    ),
                )
                for i in range(g0, g1):
                    nc.vector.tensor_scalar(
                        out=m_t[:, i - g0, :], in0=x_t[:, i - g0, :],
                        scalar1=th_all[:, i:i + 1],
                        scalar2=None, op0=ALU.is_ge,
                    )
                nc.sync.dma_start(
                    out=bass.AP(
                        m_d, g0 * PARTS * cpi,
                        [[cpi, PARTS], [PARTS * cpi, g1 - g0], [1, cpi]],
                    ),
                    in_=m_t[:, :g1 - g0, :],
                )
    nc.finalize()
    return nc


# ---------------------------------------------------------------------------
# Host: decode W, exact-float32 Otsu, threshold folding
def decode_hist(w_raw, nimg=IMGS_PER_CORE, npix=PIX):
    """w_raw [nimg, 128, 128] fp32 -> hist [nimg, 256] int64 (exact).

    Psum row 8*tau+c', col 8*sigma+c'': sum the c'==c'' diagonal blocks."""
    encA, encB = _enc_pm()
    P128 = np.round(np.asarray(w_raw, np.float64)).astype(np.int64)
    P128 = P128.reshape(nimg, 16, 8, 16, 8)  # [img, tau, c', sigma, c'']
    R = np.einsum("itcsc->its", P128)        # [img, tau, sigma]
    P = npix
    # marginals from always-true rows (tau=0 / sigma=0 columns are exact ones)
    sumB = np.where(encB[None, :], (R[:, 0, :] + P) // 2, R[:, 0, :])  # [img,16]
    sumA = np.where(encA[None, :], (R[:, :, 0] + P) // 2, R[:, :, 0])  # [img,16]
    eA = encA[None, :, None]
    eB = encB[None, None, :]
    sA = sumA[:, :, None]
    sB = sumB[:, None, :]
    W = np.where(
        ~eA & ~eB, R,
        np.where(
            eA & ~eB, (R + sB) // 2,
            np.where(~eA & eB, (R + sA) // 2, (R + 2 * sA + 2 * sB - P) // 4),
        ),
    )
    # sanity: the integer divisions above must be exact
    chk = np.where(
        ~eA & ~eB, 0,
        np.where(eA & ~eB, (R + sB) % 2,
                 np.where(~eA & eB, (R + sA) % 2, (R + 2 * sA + 2 * sB - P) % 4)),
    )
    assert not chk.any(), "non-integer decode: device histogram corrupted"
    Wp = np.zeros((nimg, 17, 17), np.int64)
    Wp[:, :16, :16] = W
    hist = (Wp[:, :16, :16] - Wp[:, 1:, :16] - Wp[:, :16, 1:] + Wp[:, 1:, 1:])
    hist = hist.reshape(nimg, 256)
    assert (hist >= 0).all() and (hist.sum(1) == P).all(), "bad histogram"
    return hist


def otsu_f32(hist):
    """Mirror the jax float32 reference exactly. hist [n, 256] int64 -> th int, bad mask."""
    f = hist.astype(np.float32)
    centers = np.arange(NBINS, dtype=np.float32)
    w1 = np.cumsum(f, axis=1, dtype=np.float32)
    total = w1[:, -1:]
    s1 = np.cumsum(f * centers, axis=1, dtype=np.float32)
    stot = s1[:, -1:]
    w2 = total - w1
    with np.errstate(divide="ignore", invalid="ignore"):
        m1 = s1 / w1
        m2 = (stot - s1) / w2
        d = m1 - m2
        var12 = (w1 * w2) * (d * d)
    nz = hist > 0
    t = np.arange(NBINS)
    vmin = np.argmax(nz, axis=1)
    vmax = NBINS - 1 - np.argmax(nz[:, ::-1], axis=1)
    valid = (t[None, :] >= vmin[:, None]) & (t[None, :] < vmax[:, None])
    var12 = np.where(valid, var12, np.float32(-1.0))
    th = np.argmax(var12, axis=1)
    th = np.where(th == 0, 1, th)
    th = np.where(th == 255, 254, th)
    bad = vmin == vmax
    return th, bad


def _min_x_for_cut(c):
    """Smallest fp32 x with fl(255*x) >= c (c integer 1..255)."""
    f255 = np.float32(255.0)
    d = np.float32(np.float64(c) / 255.0)
    # walk down while still satisfying, then ensure satisfied
    for _ in range(8):
        dn = np.nextafter(d, np.float32(-1.0), dtype=np.float32)
        if np.float32(f255 * dn) >= c:
            d = dn
        else:
            break
    while np.float32(f255 * d) < c:
        d = np.nextafter(d, np.float32(2.0), dtype=np.float32)
    return d


_CUT_TABLE = None


def cut_table():
    global _CUT_TABLE
    if _CUT_TABLE is None:
        _CUT_TABLE = np.array(
            [np.float32(0.0)] + [_min_x_for_cut(c) for c in range(1, 256)],
            dtype=np.float32,
        )
    return _CUT_TABLE


def thresholds_to_cuts(th, bad):
    """mask = (cam > th) == (x >= dth); degenerate images -> never."""
    tab = cut_table()
    dth = tab[np.asarray(th) + 1]
    return np.where(bad, np.float32(2.0), dth).astype(np.float32)


# ---------------------------------------------------------------------------
_NC_CACHE = {}


def _get_ncs():
    if "hist" not in _NC_CACHE:
        _NC_CACHE["hist"] = build_hist_nc()
        _NC_CACHE["mask"] = build_mask_nc()
    return _NC_CACHE["hist"], _NC_CACHE["mask"]


def kernel(x: np.ndarray, _profile: dict | None = None) -> np.ndarray:
    x = np.ascontiguousarray(np.asarray(x, dtype=np.float32))
    assert x.shape == (B, N, H, W_IMG)
    nc_hist, nc_mask = _get_ncs()

    bpc = B // N_CORES
    shards = [
        np.ascontiguousarray(
            x[k * bpc:(k + 1) * bpc].reshape(IMGS_PER_CORE, PARTS, CPI)
        )
        for k in range(N_CORES)
    ]
    core_ids = list(range(N_CORES))

    kwargs_a = dict(_profile.get("a", {})) if _profile else {}
    res_a = run_bass_kernel_spmd(
        nc_hist, [{"x": s} for s in shards], core_ids=core_ids, **kwargs_a
    )
    if _profile is not None:
        _profile["res_a"] = res_a

    dths = []
    for k in range(N_CORES):
        hist = decode_hist(res_a.results[k]["w_raw"])
        th, bad = otsu_f32(hist)
        dth = thresholds_to_cuts(th, bad)
        dths.append(np.repeat(dth[:, None], PARTS, axis=1).astype(np.float32))

    kwargs_b = dict(_profile.get("b", {})) if _profile else {}
    res_b = run_bass_kernel_spmd(
        nc_mask,
        [{"x": s, "dth": d} for s, d in zip(shards, dths)],
        core_ids=core_ids,
        **kwargs_b,
    )
    if _profile is not None:
        _profile["res_b"] = res_b

    out = np.empty((B, N, H, W_IMG), np.int32)
    for k in range(N_CORES):
        m = res_b.results[k]["mask"]  # [64, 128, 392] u8
        out[k * bpc:(k + 1) * bpc] = (
            m.reshape(bpc, N, H, W_IMG).astype(np.int32)
        )
    return out
